# revision 4
# baseline (speedup 1.0000x reference)
"""Trainium2 Bass kernel v4 for nn_DualContrastiveModel (GAT-style relational attention).

Math per batch b (N=256 nodes, D=128 features, 4 relation types):
    g_r[i,j] = sum_d h[i,d]*a_r[d]*h[j,d]
    scores   = leakyrelu(g)_{adj-1} where adj in {1..4}, -inf where adj==0
    alpha    = softmax(scores, axis=-1)
    out      = alpha @ h

v4 = v2 (f16 scores + fp8 DoubleRow mask-inject + DVE reduce selection)
with op-count amortization over batch QUADS:
  - all input/output DMAs quad-batched (host lays [Bs/4, P, 4, ...]), so
    ~1 DMA issue per batch instead of ~4.4 (the v2 limiter: DMA issue /
    semaphore chain time on the sync sequencer)
  - hw built on GPSIMD as ONE broadcast tensor_tensor per quad
  - prelu and exp are one ACT op per quad (fd 2048) instead of per batch
  - per-batch work stays: 8 score/inject matmuls, 2 DVE reduces, 4 output
    matmuls, reciprocal, 2 normalizing drains (both on ACT)
"""

import os
import sys

import numpy as np

for _p in ("/root/.axon_site/_ro/trn_rl_repo", "/opt/trn_rl_repo"):
    if os.path.isdir(_p) and _p not in sys.path:
        sys.path.append(_p)

_BASS_STATE = {}

BIGM = 240.0
PRESCALE = 4.0
Q = 4  # batches per quad


def _build_program(Bshard: int, repeat: int = 1):
    from contextlib import ExitStack, nullcontext

    import concourse.bacc as bacc
    import concourse.mybir as mybir
    import concourse.tile as tile

    f32 = mybir.dt.float32
    f16 = mybir.dt.float16
    fp8 = mybir.dt.float8e4
    DRmode = mybir.MatmulPerfMode.DoubleRow
    OP = mybir.AluOpType
    AF = mybir.ActivationFunctionType
    AX = mybir.AxisListType
    N, D, P = 256, 128, 128
    assert Bshard % Q == 0
    NQ = Bshard // Q

    nc = bacc.Bacc(
        "TRN2",
        target_bir_lowering=False,
        debug=False,
        enable_asserts=False,
        num_devices=8,
    )
    hT_d = nc.dram_tensor("hT16", [NQ, P, Q, N], f16, kind="ExternalInput").ap()
    cpk_d = nc.dram_tensor("cpk", [NQ, 64, Q, 2, 2, 512], fp8,
                           kind="ExternalInput").ap()
    xt_d = nc.dram_tensor("xt", [NQ, P, Q, 2, D + 1], f16,
                          kind="ExternalInput").ap()
    av_d = nc.dram_tensor("avec", [P, 4], f32, kind="ExternalInput").ap()
    ipa_d = nc.dram_tensor("ipa", [64, 2, P], fp8, kind="ExternalInput").ap()
    ipb_d = nc.dram_tensor("ipb", [64, 2, P], fp8, kind="ExternalInput").ap()
    out_d = nc.dram_tensor("out", [NQ, P, Q, 2, D], f32, kind="ExternalOutput").ap()

    with tile.TileContext(nc) as tc:
        with ExitStack() as ctx:
            ep = ctx.enter_context

            consts = ep(tc.tile_pool(name="consts", bufs=1))
            ipa = consts.tile([64, 2, P], fp8)
            nc.sync.dma_start(ipa, ipa_d)
            ipb = consts.tile([64, 2, P], fp8)
            nc.sync.dma_start(ipb, ipb_d)
            avec = consts.tile([P, 4], f32)
            nc.sync.dma_start(avec, av_d)
            nbias = consts.tile([P, 1], f32)
            nc.vector.memset(nbias, -BIGM / PRESCALE)

            hT_p = ep(tc.tile_pool(name="hT", bufs=2))
            hw_p = ep(tc.tile_pool(name="hw", bufs=2))
            cpk_p = ep(tc.tile_pool(name="cpk", bufs=2))
            xt_p = ep(tc.tile_pool(name="xt", bufs=3))
            sel_p = ep(tc.tile_pool(name="sel", bufs=2))
            pT_p = ep(tc.tile_pool(name="pT", bufs=2))
            rs_p = ep(tc.tile_pool(name="rs", bufs=6))
            ob_p = ep(tc.tile_pool(name="ob", bufs=2))

            tp0_p = ep(tc.tile_pool(name="tp0", bufs=2, space="PSUM"))
            tp1_p = ep(tc.tile_pool(name="tp1", bufs=1, space="PSUM"))
            po_p = ep(tc.tile_pool(name="po", bufs=2, space="PSUM"))

            def emit_head(q):
                st = {}
                hT = hT_p.tile([P, Q, N], f16, tag="hT", name=f"hT{q}")
                nc.sync.dma_start(hT, hT_d[q])
                cpk = cpk_p.tile([64, Q, 2, 2, 512], fp8, tag="cpk", name=f"cpk{q}")
                nc.sync.dma_start(cpk, cpk_d[q])
                xt = xt_p.tile([P, Q, 2, D + 1], f16, tag="xt", name=f"xt{q}")
                nc.sync.dma_start(xt, xt_d[q])
                # hw[p, qb, slot, i] = avec[p, slot] * hT[p, qb, i] in one op
                hw = hw_p.tile([P, Q, 4, N], f16, tag="hw", name=f"hw{q}")
                nc.gpsimd.tensor_tensor(
                    hw,
                    hT.rearrange("p q (o n) -> p q o n", o=1).broadcast_to(
                        [P, Q, 4, N]),
                    avec.rearrange("p (o s u) -> p o s u", o=1, u=1).broadcast_to(
                        [P, Q, 4, N]),
                    op=OP.mult,
                )
                st["hT"], st["hw"], st["cpk"], st["xt"] = hT, hw, cpk, xt
                st["sel"] = sel_p.tile([P, Q, 2, N], f32, tag="sel", name=f"sel{q}")
                return st

            def emit_mm(q, qb, st):
                # t[j, J, slot, i], slot order (r0, r2, r1, r3); per-bank MMs
                tps = []
                hwf = st["hw"][:, qb].rearrange("p r i -> p (r i)")
                for J, pool in ((0, tp0_p), (1, tp1_p)):
                    tp = pool.tile([P, 4, N], f32, tag=f"tp{J}",
                                   name=f"tp{J}_{q}_{qb}")
                    tps.append(tp)
                    lhs_h = st["hT"][:, qb, J * P:(J + 1) * P]
                    cJ = st["cpk"][:, qb, :, J, :]
                    for bank in range(2):
                        nc.tensor.matmul(
                            tp[:, 2 * bank:2 * bank + 2, :].rearrange(
                                "p r i -> p (r i)"),
                            lhsT=lhs_h,
                            rhs=hwf[:, bank * 512:(bank + 1) * 512],
                            start=True, stop=False,
                        )
                    nc.tensor.matmul(
                        tp[:, 0:2, :].rearrange("p r i -> p (r i)"),
                        lhsT=ipa, rhs=cJ,
                        start=False, stop=True, perf_mode=DRmode,
                        skip_group_check=True,
                    )
                    nc.tensor.matmul(
                        tp[:, 2:4, :].rearrange("p r i -> p (r i)"),
                        lhsT=ipb, rhs=cJ,
                        start=False, stop=True, perf_mode=DRmode,
                        skip_group_check=True,
                    )
                # selection immediately (frees PSUM)
                for J in range(2):
                    nc.vector.tensor_reduce(
                        st["sel"][:, qb, J, :],
                        tps[J].rearrange("p r i -> p i r"),
                        axis=AX.X, op=OP.max,
                    )

            def emit_act(q, st):
                # one prelu + one exp over the whole quad (fd 2048)
                ul = pT_p.tile([P, Q, 2, N], f16, tag="ul", name=f"ul{q}")
                nc.scalar.activation(
                    ul, st["sel"], AF.Prelu,
                    bias=nbias, scale=1.0 / PRESCALE, alpha=0.2,
                )
                pT = pT_p.tile([P, Q, 2, N], f16, tag="pT", name=f"pT{q}")
                nc.scalar.activation(pT, ul, AF.Exp)
                st["pT"] = pT

            def emit_out(q, qb, st):
                po = po_p.tile([P, 2, D + 1], f32, tag="po", name=f"po{q}_{qb}")
                for I in range(2):
                    for J in range(2):
                        nc.tensor.matmul(
                            po[:, I, :],
                            lhsT=st["pT"][:, qb, J, I * P:(I + 1) * P],
                            rhs=st["xt"][:, qb, J, :],
                            start=(J == 0), stop=(J == 1),
                        )
                rs = rs_p.tile([P, 2], f32, tag="rs", name=f"rs{q}_{qb}")
                nc.vector.reciprocal(rs, po[:, :, D])
                ob = st["ob"]
                for I in range(2):
                    nc.scalar.activation(
                        ob[:, qb, I, :], po[:, I, 0:D], AF.Copy,
                        bias=0.0, scale=rs[:, I:I + 1],
                    )

            def emit_ostore(q, st):
                nc.sync.dma_start(out_d[q], st["ob"])

            loop_cm = tc.For_i(0, repeat, 1) if repeat > 1 else nullcontext()
            with loop_cm:
                sts = {}
                for q in range(NQ + 3):
                    if q < NQ:
                        sts[q] = emit_head(q)
                    if 1 <= q <= NQ:
                        for qb in range(Q):
                            emit_mm(q - 1, qb, sts[q - 1])
                    if 2 <= q <= NQ + 1:
                        emit_act(q - 2, sts[q - 2])
                        sts[q - 2]["ob"] = ob_p.tile(
                            [P, Q, 2, D], f32, tag="ob", name=f"ob{q - 2}")
                    if q >= 3:
                        for qb in range(Q):
                            emit_out(q - 3, qb, sts[q - 3])
                        emit_ostore(q - 3, sts.pop(q - 3))

    nc.compile()
    return nc


def _get_program(Bshard: int):
    key = ("prog", Bshard)
    if key not in _BASS_STATE:
        _BASS_STATE[key] = _build_program(Bshard)
    return _BASS_STATE[key]


def _host_pack(hidden: np.ndarray, adj: np.ndarray, apack: np.ndarray):
    """Build quad-laid hT16, cpk, xt, avec, ipa, ipb. hidden [B,N,D] f32."""
    import ml_dtypes

    e4 = ml_dtypes.float8_e4m3fn
    B, N, D = hidden.shape

    # hT16 [B/Q, P, Q, N]
    hT = hidden.transpose(0, 2, 1).astype(np.float16)  # [B, d, j]
    hT16 = np.ascontiguousarray(hT.reshape(-1, Q, D, N).transpose(0, 2, 1, 3))

    # cpk [B/Q, 64, Q, 2(ko), 2(J), 512(pl,i)], j = J*128 + 2*ki + ko
    adjT = adj.transpose(0, 2, 1)  # [B, j, i]
    pl0 = (adjT == 1).astype(np.float32) - 8.0 * (adjT == 2)
    pl1 = (adjT == 3).astype(np.float32) - 8.0 * (adjT == 4)
    pl = np.stack([pl0, pl1], axis=2)  # [B, j, pl, i]
    plJ = pl.reshape(B, 2, 64, 2, 2 * N)  # [B, J, ki, ko, (pl,i)]
    cpk = np.ascontiguousarray(
        plJ.transpose(0, 2, 3, 1, 4).reshape(-1, Q, 64, 2, 2, 512)
        .transpose(0, 2, 1, 3, 4, 5).astype(e4)
    )

    # xt [B/Q, P, Q, 2, D+1]
    xt = np.concatenate(
        [hidden, np.ones((B, N, 1), np.float32)], axis=2
    ).reshape(B, 2, 128, D + 1).transpose(0, 2, 1, 3)  # [B, p, J, D+1]
    xt = np.ascontiguousarray(
        xt.reshape(-1, Q, 128, 2, D + 1).transpose(0, 2, 1, 3, 4)
        .astype(np.float16))

    avec = np.ascontiguousarray(
        (PRESCALE * apack[:, (0, 2, 1, 3)]).astype(np.float32))

    ident = np.zeros((64, 2, 128), dtype=np.float32)
    ki = np.arange(64)
    ident[ki, 0, 2 * ki] = 1.0
    ident[ki, 1, 2 * ki + 1] = 1.0
    ipa = np.ascontiguousarray((BIGM * ident).astype(e4))
    ipb = np.ascontiguousarray((-BIGM / 8.0 * ident).astype(e4))
    return hT16, cpk, xt, avec, ipa, ipb


def kernel(hidden: np.ndarray, adj: np.ndarray, a_0, a_1, a_2, a_3) -> np.ndarray:
    from concourse import bass_utils

    B, N, D = hidden.shape
    NCORES = 8
    assert B % NCORES == 0
    Bs = B // NCORES

    apack = np.ascontiguousarray(
        np.concatenate([a_0, a_1, a_2, a_3], axis=1).astype(np.float32)
    )
    hidden = np.ascontiguousarray(hidden, dtype=np.float32)
    hT16, cpk, xt, avec, ipa, ipb = _host_pack(hidden, adj, apack)
    nq = Bs // Q

    nc = _get_program(Bs)
    in_maps = [
        {
            "hT16": hT16[c * nq:(c + 1) * nq],
            "cpk": cpk[c * nq:(c + 1) * nq],
            "xt": xt[c * nq:(c + 1) * nq],
            "avec": avec,
            "ipa": ipa,
            "ipb": ipb,
        }
        for c in range(NCORES)
    ]
    res = bass_utils.run_bass_kernel_spmd(
        nc,
        in_maps,
        core_ids=list(range(NCORES)),
        trace=bool(int(os.environ.get("KERNEL_TRACE", "0"))),
    )
    _BASS_STATE["last_result"] = res
    out = np.concatenate([r["out"] for r in res.results], axis=0)
    # [B/Q, p, qb, I, d] -> [B, i=I*128+p, d]
    out = out.transpose(0, 2, 3, 1, 4).reshape(B, N, D)
    return np.ascontiguousarray(out)


# revision 5
# speedup vs baseline: 1.3317x; 1.3317x over previous
"""Trainium2 Bass kernel v2 for nn_DualContrastiveModel (GAT-style relational attention).

Math per batch b (N=256 nodes, D=128 features, 4 relation types):
    g_r[i,j] = sum_d h[i,d]*a_r[d]*h[j,d]
    scores   = leakyrelu(g)_{adj-1} where adj in {1..4}, -inf where adj==0
    alpha    = softmax(scores, axis=-1)
    out      = alpha @ h

v2 strategy (vs the v1 baseline at ~3.07us/batch):
  - j-major scores (g is symmetric, masks host-transposed): no PE transposes
    anywhere; exp output feeds the output matmul as lhsT directly.
  - scores in f16 (4 matmuls of N=512 per batch, two relations per PSUM
    bank); relation planes are ordered (r0, r2, r1, r3) so each mask inject
    covers one full bank with a single stationary.
  - mask inject in fp8 e4m3 DoubleRow mode: host packs signed planes
    pl0 = m1 - 8*m2, pl1 = m3 - 8*m4 (exact in e4m3) with j-pair rows
    [64, 2, ...]; stationaries are j-pair identities scaled +240 / -30.
    Packing contamination is strictly negative on non-selected planes, so
    max-selection is exact; one inject = one N=512 DoubleRow matmul.
  - selection: DVE max-reduce over the 4 relation planes per J-half.
  - prelu folds the x4 pre-scale and -60 boost bias; exp writes f16.
  - hw = 4*a_r (x) h^T built on GPSIMD (tensor_scalar with per-partition
    a-vector), keeping DVE/ACT free for selection/activation.
  - row-sum via ones-column in xt; normalization applied during the
    mandatory PSUM->SBUF drains (one on ACT, one on DVE); recip on DVE.
  - PSUM: tp-J0 pool bufs=2 (4 banks) + tp-J1 bufs=1 (2 banks) +
    po bufs=2 (2 banks) = exactly 8 banks.
"""

import os
import sys

import numpy as np

for _p in ("/root/.axon_site/_ro/trn_rl_repo", "/opt/trn_rl_repo"):
    if os.path.isdir(_p) and _p not in sys.path:
        sys.path.append(_p)

_BASS_STATE = {}

BIGM = 240.0  # mask boost in t-units; e4m3-exact; boost in g-units = 240/4 = 60
PRESCALE = 4.0  # folded into avec; prelu applies 1/PRESCALE


def _build_program(Bshard: int, repeat: int = 1):
    from contextlib import ExitStack, nullcontext

    import concourse.bacc as bacc
    import concourse.mybir as mybir
    import concourse.tile as tile

    f32 = mybir.dt.float32
    f16 = mybir.dt.float16
    fp8 = mybir.dt.float8e4
    DRmode = mybir.MatmulPerfMode.DoubleRow
    OP = mybir.AluOpType
    AF = mybir.ActivationFunctionType
    AX = mybir.AxisListType
    N, D, P = 256, 128, 128

    nc = bacc.Bacc(
        "TRN2",
        target_bir_lowering=False,
        debug=False,
        enable_asserts=False,
        num_devices=8,
    )
    hT_d = nc.dram_tensor("hT16", [Bshard, P, N], f16, kind="ExternalInput").ap()
    cpk_d = nc.dram_tensor("cpk", [Bshard, 64, 2, 2, 512], fp8,
                           kind="ExternalInput").ap()
    xt_d = nc.dram_tensor("xt", [Bshard, P, 2, D + 1], f16, kind="ExternalInput").ap()
    av_d = nc.dram_tensor("avec", [P, 4], f32, kind="ExternalInput").ap()
    ipa_d = nc.dram_tensor("ipa", [64, 2, P], fp8, kind="ExternalInput").ap()
    ipb_d = nc.dram_tensor("ipb", [64, 2, P], fp8, kind="ExternalInput").ap()
    out_d = nc.dram_tensor("out", [Bshard, P, 2, D], f32, kind="ExternalOutput").ap()

    with tile.TileContext(nc) as tc:
        with ExitStack() as ctx:
            ep = ctx.enter_context

            consts = ep(tc.tile_pool(name="consts", bufs=1))
            ipa = consts.tile([64, 2, P], fp8)
            nc.sync.dma_start(ipa, ipa_d)
            ipb = consts.tile([64, 2, P], fp8)
            nc.sync.dma_start(ipb, ipb_d)
            avec = consts.tile([P, 4], f32)
            nc.sync.dma_start(avec, av_d)
            nbias = consts.tile([P, 1], f32)
            nc.vector.memset(nbias, -BIGM / PRESCALE)

            hT_p = ep(tc.tile_pool(name="hT", bufs=4))
            hw_p = ep(tc.tile_pool(name="hw", bufs=4))
            cpk_p = ep(tc.tile_pool(name="cpk", bufs=4))
            xt_p = ep(tc.tile_pool(name="xt", bufs=5))
            sel_p = ep(tc.tile_pool(name="sel", bufs=3))
            ul_p = ep(tc.tile_pool(name="ul", bufs=3))
            pT_p = ep(tc.tile_pool(name="pT", bufs=3))
            rs_p = ep(tc.tile_pool(name="rs", bufs=3))
            ob_p = ep(tc.tile_pool(name="ob", bufs=3))

            tp0_p = ep(tc.tile_pool(name="tp0", bufs=2, space="PSUM"))
            tp1_p = ep(tc.tile_pool(name="tp1", bufs=1, space="PSUM"))
            po_p = ep(tc.tile_pool(name="po", bufs=2, space="PSUM"))

            def emit_head(b):
                st = {}
                hT = hT_p.tile([P, N], f16, tag="hT", name=f"hT{b}")
                nc.sync.dma_start(hT, hT_d[b])
                cpk = cpk_p.tile([64, 2, 2, 512], fp8, tag="cpk", name=f"cpk{b}")
                nc.sync.dma_start(cpk, cpk_d[b])
                xt = xt_p.tile([P, 2, D + 1], f16, tag="xt", name=f"xt{b}")
                nc.sync.dma_start(xt, xt_d[b])
                # hw slots ordered (r0, r2, r1, r3): avec cols 0,2,1,3
                hw = hw_p.tile([P, 4, N], f16, tag="hw", name=f"hw{b}")
                for s, r in enumerate((0, 2, 1, 3)):
                    nc.gpsimd.tensor_tensor(
                        hw[:, s, :], hT,
                        avec[:, r:r + 1].broadcast_to([P, N]), op=OP.mult,
                    )
                st["hT"], st["hw"], st["cpk"], st["xt"] = hT, hw, cpk, xt
                return st

            def emit_mm(b, st):
                # t[j, J, slot, i], slot order (r0, r2, r1, r3); per-bank MMs
                tps = []
                hwf = st["hw"].rearrange("p r i -> p (r i)")
                for J, pool in ((0, tp0_p), (1, tp1_p)):
                    tp = pool.tile([P, 4, N], f32, tag=f"tp{J}", name=f"tp{J}_{b}")
                    tps.append(tp)
                    lhs_h = st["hT"][:, J * P:(J + 1) * P]
                    cJ = st["cpk"][:, :, J, :]
                    for bank in range(2):
                        nc.tensor.matmul(
                            tp[:, 2 * bank:2 * bank + 2, :].rearrange(
                                "p r i -> p (r i)"),
                            lhsT=lhs_h,
                            rhs=hwf[:, bank * 512:(bank + 1) * 512],
                            start=True, stop=False,
                        )
                    nc.tensor.matmul(
                        tp[:, 0:2, :].rearrange("p r i -> p (r i)"),
                        lhsT=ipa, rhs=cJ,
                        start=False, stop=True, perf_mode=DRmode,
                        skip_group_check=True,
                    )
                    nc.tensor.matmul(
                        tp[:, 2:4, :].rearrange("p r i -> p (r i)"),
                        lhsT=ipb, rhs=cJ,
                        start=False, stop=True, perf_mode=DRmode,
                        skip_group_check=True,
                    )
                st["tps"] = tps

            def emit_sel(b, st):
                sel = sel_p.tile([P, 2, N], f32, tag="sel", name=f"sel{b}")
                for J in range(2):
                    nc.vector.tensor_reduce(
                        sel[:, J, :],
                        st["tps"][J].rearrange("p r i -> p i r"),
                        axis=AX.X, op=OP.max,
                    )
                st["sel"] = sel

            def emit_act(b, st):
                ul = ul_p.tile([P, 2, N], f16, tag="ul", name=f"ul{b}")
                nc.scalar.activation(
                    ul, st["sel"], AF.Prelu,
                    bias=nbias, scale=1.0 / PRESCALE, alpha=0.2,
                )
                pT = pT_p.tile([P, 2, N], f16, tag="pT", name=f"pT{b}")
                nc.scalar.activation(pT, ul, AF.Exp)
                st["pT"] = pT

            def emit_out(b, st):
                po = po_p.tile([P, 2, D + 1], f32, tag="po", name=f"po{b}")
                for I in range(2):
                    for J in range(2):
                        nc.tensor.matmul(
                            po[:, I, :],
                            lhsT=st["pT"][:, J, I * P:(I + 1) * P],
                            rhs=st["xt"][:, J, :],
                            start=(J == 0), stop=(J == 1),
                        )
                rs = rs_p.tile([P, 2], f32, tag="rs", name=f"rs{b}")
                nc.vector.reciprocal(rs, po[:, :, D])
                ob = ob_p.tile([P, 2, D], f32, tag="ob", name=f"ob{b}")
                nc.scalar.activation(
                    ob[:, 0, :], po[:, 0, 0:D], AF.Copy, bias=0.0, scale=rs[:, 0:1]
                )
                nc.scalar.activation(
                    ob[:, 1, :], po[:, 1, 0:D], AF.Copy, bias=0.0, scale=rs[:, 1:2]
                )
                nc.sync.dma_start(out_d[b], ob)

            loop_cm = tc.For_i(0, repeat, 1) if repeat > 1 else nullcontext()
            with loop_cm:
                sts = {}
                for b in range(Bshard + 4):
                    if b < Bshard:
                        sts[b] = emit_head(b)
                    if 1 <= b <= Bshard:
                        emit_mm(b - 1, sts[b - 1])
                    if 2 <= b <= Bshard + 1:
                        emit_sel(b - 2, sts[b - 2])
                    if 3 <= b <= Bshard + 2:
                        emit_act(b - 3, sts[b - 3])
                    if b >= 4:
                        emit_out(b - 4, sts.pop(b - 4))

    nc.compile()
    return nc


def _get_program(Bshard: int):
    key = ("prog", Bshard)
    if key not in _BASS_STATE:
        _BASS_STATE[key] = _build_program(Bshard)
    return _BASS_STATE[key]


def _host_pack(hidden: np.ndarray, adj: np.ndarray, apack: np.ndarray):
    """Build hT16, cpk, xt, avec, ipa, ipb host-side. hidden [B,N,D] f32."""
    import ml_dtypes

    e4 = ml_dtypes.float8_e4m3fn
    B, N, D = hidden.shape

    hT16 = np.ascontiguousarray(hidden.transpose(0, 2, 1).astype(np.float16))

    # packed signed mask planes, j-major (transposed), j-pair rows:
    #   pl0 = (adjT==1) - 8*(adjT==2)  -> bank0 via +240, bank1 via -30
    #   pl1 = (adjT==3) - 8*(adjT==4)
    # cpk[b, ki, ko, J, (pl, i)] with j = J*128 + 2*ki + ko
    adjT = adj.transpose(0, 2, 1)  # [B, j, i]
    pl0 = (adjT == 1).astype(np.float32) - 8.0 * (adjT == 2)
    pl1 = (adjT == 3).astype(np.float32) - 8.0 * (adjT == 4)
    pl = np.stack([pl0, pl1], axis=2)  # [B, j, pl, i]
    plJ = pl.reshape(B, 2, 64, 2, 2, N)  # [B, J, ki, ko, pl, i]
    cpk = np.ascontiguousarray(
        plJ.transpose(0, 2, 3, 1, 4, 5).reshape(B, 64, 2, 2, 512).astype(e4)
    )

    xt = np.concatenate(
        [hidden, np.ones((B, N, 1), np.float32)], axis=2
    ).reshape(B, 2, 128, D + 1).transpose(0, 2, 1, 3)
    xt = np.ascontiguousarray(xt.astype(np.float16))

    avec = np.ascontiguousarray((PRESCALE * apack).astype(np.float32))

    ident = np.zeros((64, 2, 128), dtype=np.float32)
    ki = np.arange(64)
    ident[ki, 0, 2 * ki] = 1.0
    ident[ki, 1, 2 * ki + 1] = 1.0
    ipa = np.ascontiguousarray((BIGM * ident).astype(e4))
    ipb = np.ascontiguousarray((-BIGM / 8.0 * ident).astype(e4))
    return hT16, cpk, xt, avec, ipa, ipb


def kernel(hidden: np.ndarray, adj: np.ndarray, a_0, a_1, a_2, a_3) -> np.ndarray:
    from concourse import bass_utils

    B, N, D = hidden.shape
    NCORES = 8
    assert B % NCORES == 0
    Bs = B // NCORES

    apack = np.ascontiguousarray(
        np.concatenate([a_0, a_1, a_2, a_3], axis=1).astype(np.float32)
    )
    hidden = np.ascontiguousarray(hidden, dtype=np.float32)
    hT16, cpk, xt, avec, ipa, ipb = _host_pack(hidden, adj, apack)

    nc = _get_program(Bs)
    in_maps = [
        {
            "hT16": hT16[c * Bs:(c + 1) * Bs],
            "cpk": cpk[c * Bs:(c + 1) * Bs],
            "xt": xt[c * Bs:(c + 1) * Bs],
            "avec": avec,
            "ipa": ipa,
            "ipb": ipb,
        }
        for c in range(NCORES)
    ]
    res = bass_utils.run_bass_kernel_spmd(
        nc,
        in_maps,
        core_ids=list(range(NCORES)),
        trace=bool(int(os.environ.get("KERNEL_TRACE", "0"))),
    )
    _BASS_STATE["last_result"] = res
    out = np.concatenate([r["out"] for r in res.results], axis=0)
    # [B, p, I, d] -> [B, i=I*128+p, d]
    return np.ascontiguousarray(out.transpose(0, 2, 1, 3).reshape(B, N, D))
